# revision 2
# baseline (speedup 1.0000x reference)
"""ConvGRU Trainium2 kernel, v2: merged [zbar|r] sigmoid.

video [B=2, T=16, C=128, H=64, W=64] f32; 1x1-conv GRU over T.
Sharding: data-parallel over (B x H/16) -> 8 cores, each core owns
P = 16*64 = 1024 pixels for all T; weights replicated.

The Scalar (ACT) engine paces the kernel: 3*C*P sigmoid/tanh elements
per step.  v1 streamed 6 ACTIVATE ops x ~687ns = 4.13us/step; each op
pays ~260ns of fixed overhead (PSUM access latency + seq/dispatch), so
wider ops win.  v2 merges each group's z and r sigmoids into ONE
[C, 2*PG] ACTIVATE:

  - The z-gate is computed as zbar = sigmoid(-(pre_z + bz)) by negating
    Wz/bz on the HOST, so both halves of the merged op share scale=+1
    and bias=0.
  - The per-half biases (-bz | +br) cannot ride the ACT bias port (it is
    per-partition); instead the zr PSUM tile is PRE-FILLED with them by
    the idle Vector engine (tensor_copy from a const SBUF tile), and all
    matmuls accumulate with start=False (skip_group_check silences the
    sim's accumulation-group tracking; HW adds onto the filled bank).
  - ACT stream/step: sig_a [C,1024] 1114ns, sig_b 1114, tanh_a 687,
    tanh_b 687 = 3.60us vs 4.13us in v1.

Blend keeps v1's short-tail structure, in u/w form:
    u = zbar*h, w = (zbar-1)*c = -z*c, h' = u - w
    r-close(t+1) = Wrh@u + (-Wrh)@w   (split: leaves the critical path)
    z-close(t+1) = (-Wzh)@h'          (single: zbar halves have slack)
The group-b zr PSUM tile is double-buffered so its bias fill lands in
the idle DVE window at period start; group-a's fill (single-buffered)
is split into halves interleaved with the critical rh/u ops.

Startup: x0 is split into quarters over the sync+scalar HWDGE queues,
weights ride gpsimd/sync ordered by first use (t0 needs only -Wzx,Whx),
and the ACT warmup sigmoid uses a const bias so the activation-table
load (1283ns) runs during the DMA ramp instead of on the t0 path.

Numerics: fp16 matmul inputs/gates/state, fp32 PSUM accum, biases
accumulated in fp32 PSUM (z/r) or via fp32 ACT bias AP (tanh).
"""

import os
import sys

import numpy as np

B, T, C, H, W = 2, 16, 128, 64, 64
NCORES = 8
HQ = H // 4          # 16 rows of H per core (4 H-slices x 2 batches = 8 cores)
P = HQ * W           # 1024 pixels per core
G = 2                # pixel groups per step (independent recurrence chains)
PG = P // G          # 512 pixels per group

_PROG = None


def _ensure_paths():
    for p in ("/opt/trn_rl_repo",):
        if p not in sys.path and os.path.isdir(p):
            sys.path.append(p)


def _build():
    _ensure_paths()
    import concourse.bacc as bacc
    import concourse.tile as tile
    from concourse import mybir

    f32 = mybir.dt.float32
    f16 = mybir.dt.float16
    AF = mybir.ActivationFunctionType
    OP = mybir.AluOpType

    nc = bacc.Bacc(
        "TRN2", target_bir_lowering=False, debug=False, num_devices=NCORES
    )
    x_dram = nc.dram_tensor("x_seq", [T, C, P], f16, kind="ExternalInput")
    w_dram = nc.dram_tensor("wmats", [C, 7 * C], f16, kind="ExternalInput")
    b_dram = nc.dram_tensor("biases", [C, 4], f32, kind="ExternalInput")
    o_dram = nc.dram_tensor("out_seq", [T, C, P], f16, kind="ExternalOutput")

    x_ap = x_dram.ap()
    w_ap = w_dram.ap()
    b_ap = b_dram.ap()
    o_ap = o_dram.ap()

    # weight blocks (negations applied on host):
    #   0 WZXN = -Wz_x.T   1 WHX = Wh_x.T   2 WRX = Wr_x.T
    #   3 WZHN = -Wz_h.T   4 WRH = Wr_h.T   5 WHH = Wh_h.T   6 WRHN = -Wr_h.T
    WZXN, WHX, WRX, WZHN, WRH, WHH, WRHN = range(7)
    # bias cols: [bz, br, bh, -bz]

    Q = PG // 2  # x0 DMA quarter

    with tile.TileContext(nc) as tc:
        with (
            tc.tile_pool(name="consts", bufs=1) as consts,
            tc.tile_pool(name="xin", bufs=4) as xpool,
            tc.tile_pool(name="state", bufs=4) as spool,
            tc.tile_pool(name="work", bufs=3) as wk,
            tc.tile_pool(name="ps", bufs=1, space="PSUM") as ps,
        ):
            wt = consts.tile([C, 7 * C], f16)
            bt = consts.tile([C, 4], f32)
            btile = consts.tile([C, 2 * PG], f32)  # [-bz | +br] fill source

            def wslice(i):
                return wt[:, i * C : (i + 1) * C]

            def load_x(t):
                xt = xpool.tile([C, P], f16, tag="x")
                nc.sync.dma_start(xt[:], x_ap[t])
                return xt

            # ---- startup DMAs, ordered by first use ----
            # sync:   bt, x0q0, x0q1, w-mid, x2
            # scalar: x0q2, x0q3   (ACT then free for the table-load warmup)
            # gpsimd: w-lo, x1h0, x1h1, w-hi
            nc.sync.dma_start(bt[:], b_ap[:])
            x0t = xpool.tile([C, P], f16, tag="x", name="x0t")
            x1t = xpool.tile([C, P], f16, tag="x", name="x1t")
            nc.sync.dma_start(x0t[:, 0:Q], x_ap[0, :, 0:Q])
            nc.sync.dma_start(x0t[:, Q : 2 * Q], x_ap[0, :, Q : 2 * Q])
            nc.scalar.dma_start(x0t[:, 2 * Q : 3 * Q], x_ap[0, :, 2 * Q : 3 * Q])
            nc.scalar.dma_start(x0t[:, 3 * Q :], x_ap[0, :, 3 * Q :])
            nc.gpsimd.dma_start(wt[:, : 2 * C], w_ap[:, : 2 * C])
            nc.sync.dma_start(wt[:, 2 * C : 6 * C], w_ap[:, 2 * C : 6 * C])
            nc.gpsimd.dma_start(x1t[:, :PG], x_ap[1, :, :PG])
            nc.gpsimd.dma_start(x1t[:, PG:], x_ap[1, :, PG:])
            nc.gpsimd.dma_start(wt[:, 6 * C :], w_ap[:, 6 * C :])
            x_tiles = {0: x0t, 1: x1t}
            x_tiles[2] = load_x(2)

            # ---- PE warmup (clock-gate ramp) + ACT table preload, all on
            #      memset/const inputs so they ride the DMA ramp ----
            warm16 = wk.tile([C, PG], f16, tag="warm")
            nc.vector.memset(warm16[:], 0.0)
            nc.vector.memset(btile[:], 0.0)
            cw = [None, None]
            for g in range(G):
                cw[g] = ps.tile([C, PG], f32, tag=f"c_{g}", bufs=1,
                                name=f"cwarm_{g}")
            for i in range(6):
                nc.tensor.matmul(
                    cw[i % 2][:], warm16[:, :C], warm16[:],
                    start=True, stop=True,
                )
            wtmp = wk.tile([C, PG], f16, tag="scratch")
            nc.scalar.activation(wtmp[:], cw[0][:], AF.Sigmoid)

            # ---- bias fill source tile: [-bz | +br] ----
            nc.vector.tensor_scalar_add(btile[:, :PG], btile[:, :PG], bt[:, 3:4])
            zr0 = [None, None]
            zr0[0] = ps.tile([C, 2 * PG], f32, tag="zr_0", bufs=1, name="zr0_a")
            zr0[1] = ps.tile([C, 2 * PG], f32, tag="zr_1", bufs=2, name="zr0_b")
            # t0 uses only the z halves; fill them before the r-half build
            nc.vector.tensor_copy(zr0[0][:, :PG], btile[:, :PG])
            nc.vector.tensor_copy(zr0[1][:, :PG], btile[:, :PG])
            nc.vector.tensor_scalar_add(btile[:, PG:], btile[:, PG:], bt[:, 1:2])

            # ---- t = 0: h0 == 0 -> h(0) = z0 * c0 ----
            cp0 = [None, None]
            for g in range(G):
                xs = x0t[:, g * PG : (g + 1) * PG]
                nc.tensor.matmul(
                    zr0[g][:, :PG], wslice(WZXN), xs,
                    start=False, stop=True, skip_group_check=True,
                )
                cp = ps.tile([C, PG], f32, tag=f"c_{g}", bufs=1, name=f"cp0_{g}")
                nc.tensor.matmul(cp[:], wslice(WHX), xs, start=True, stop=True)
                cp0[g] = cp
            z016 = [None, None]
            for g in range(G):
                # z half = -(pre_z + bz); scale=-1 -> z0
                zt = wk.tile([C, PG], f16, tag=f"z0_{g}")
                nc.scalar.activation(
                    zt[:], zr0[g][:, :PG], AF.Sigmoid, scale=-1.0
                )
                z016[g] = zt
            h16 = [None, None]
            for g in range(G):
                ct = wk.tile([C, PG], f16, tag=f"c16_{g}")
                nc.scalar.activation(ct[:], cp0[g][:], AF.Tanh, bias=bt[:, 2:3])
                ht = spool.tile([C, PG], f16, tag=f"h16_{g}")
                nc.vector.tensor_mul(ht[:], z016[g][:], ct[:])
                h16[g] = ht
                nc.gpsimd.dma_start(o_ap[0, :, g * PG : (g + 1) * PG], ht[:])

            # ---- t=1 prep: fills, openers @x1, single closers @h0 ----
            zr_t = [None, None]
            cp_t = [None, None]
            zr_t[0] = ps.tile([C, 2 * PG], f32, tag="zr_0", bufs=1, name="zr1_a")
            zr_t[1] = ps.tile([C, 2 * PG], f32, tag="zr_1", bufs=2, name="zr1_b")
            for g in range(G):
                nc.vector.tensor_copy(zr_t[g][:], btile[:])
                xs = x1t[:, g * PG : (g + 1) * PG]
                nc.tensor.matmul(
                    zr_t[g][:, :PG], wslice(WZXN), xs,
                    start=False, stop=False, skip_group_check=True,
                )
                nc.tensor.matmul(
                    zr_t[g][:, PG:], wslice(WRX), xs,
                    start=False, stop=False, skip_group_check=True,
                )
                nc.tensor.matmul(
                    zr_t[g][:, :PG], wslice(WZHN), h16[g][:],
                    start=False, stop=True, skip_group_check=True,
                )
                nc.tensor.matmul(
                    zr_t[g][:, PG:], wslice(WRH), h16[g][:],
                    start=False, stop=True, skip_group_check=True,
                )
                cp = ps.tile([C, PG], f32, tag=f"c_{g}", bufs=1, name=f"cp1_{g}")
                nc.tensor.matmul(cp[:], wslice(WHX), xs, start=True, stop=False)
                cp_t[g] = cp

            # ---- steady steps t = 1..T-1 ----
            a, b = 0, 1
            for t in range(1, T):
                x_next = x_tiles.get(t + 1)
                last = x_next is None

                # ACT: merged [zbar | r] sigmoids (bias pre-filled in PSUM)
                zbr = [None, None]
                for g in (a, b):
                    zz = wk.tile([C, 2 * PG], f16, tag=f"zbr_{g}")
                    nc.scalar.activation(zz[:], zr_t[g][:], AF.Sigmoid)
                    zbr[g] = zz

                # next-step zr tiles + bias fills (DVE):
                #   zr_b is double-buffered -> its fill rides the idle
                #   window at period start; zr_a's fill splits around the
                #   critical rh_a/u_a ops.
                zr_next = [None, None]
                if not last:
                    zrb = ps.tile([C, 2 * PG], f32, tag="zr_1", bufs=2,
                                  name="zrn_b")
                    nc.vector.tensor_copy(zrb[:], btile[:])
                    zr_next[b] = zrb
                    zra = ps.tile([C, 2 * PG], f32, tag="zr_0", bufs=1,
                                  name="zrn_a")
                    nc.vector.tensor_copy(zra[:, PG:], btile[:, PG:])
                    zr_next[a] = zra

                rh_a = wk.tile([C, PG], f16, tag="rh_0")
                nc.vector.tensor_mul(rh_a[:], zbr[a][:, PG:], h16[a][:])
                u_a = wk.tile([C, PG], f16, tag="u_0")
                nc.vector.tensor_mul(u_a[:], zbr[a][:, :PG], h16[a][:])
                if not last:
                    nc.vector.tensor_copy(zr_next[a][:, :PG], btile[:, :PG])

                # PE: c-close a, r-u-close a, r-opener a
                nc.tensor.matmul(
                    cp_t[a][:], wslice(WHH), rh_a[:], start=False, stop=True
                )
                if not last:
                    nc.tensor.matmul(
                        zr_next[a][:, PG:], wslice(WRH), u_a[:],
                        start=False, stop=False, skip_group_check=True,
                    )
                    nc.tensor.matmul(
                        zr_next[a][:, PG:], wslice(WRX), x_next[:, :PG],
                        start=False, stop=False, skip_group_check=True,
                    )

                # ACT: tanh a
                c16a = wk.tile([C, PG], f16, tag="c16_0")
                nc.scalar.activation(c16a[:], cp_t[a][:], AF.Tanh,
                                     bias=bt[:, 2:3])

                rh_b = wk.tile([C, PG], f16, tag="rh_1")
                nc.vector.tensor_mul(rh_b[:], zbr[b][:, PG:], h16[b][:])
                u_b = wk.tile([C, PG], f16, tag="u_1")
                nc.vector.tensor_mul(u_b[:], zbr[b][:, :PG], h16[b][:])

                # PE: c-close b, z-opener a
                nc.tensor.matmul(
                    cp_t[b][:], wslice(WHH), rh_b[:], start=False, stop=True
                )
                if not last:
                    nc.tensor.matmul(
                        zr_next[a][:, :PG], wslice(WZXN), x_next[:, :PG],
                        start=False, stop=False, skip_group_check=True,
                    )

                # ACT: tanh b
                c16b = wk.tile([C, PG], f16, tag="c16_1")
                nc.scalar.activation(c16b[:], cp_t[b][:], AF.Tanh,
                                     bias=bt[:, 2:3])

                # blend a: w = (zbar-1)*c, h' = u - w; closers ride w/h'
                w_a = wk.tile([C, PG], f16, tag="w_0")
                nc.vector.scalar_tensor_tensor(
                    w_a[:], zbr[a][:, :PG], 1.0, c16a[:],
                    OP.subtract, OP.mult,
                )
                n_a = spool.tile([C, PG], f16, tag="h16_0")
                nc.vector.tensor_sub(n_a[:], u_a[:], w_a[:])
                if not last:
                    nc.tensor.matmul(
                        zr_next[a][:, PG:], wslice(WRHN), w_a[:],
                        start=False, stop=True, skip_group_check=True,
                    )
                    nc.tensor.matmul(
                        zr_next[a][:, :PG], wslice(WZHN), n_a[:],
                        start=False, stop=True, skip_group_check=True,
                    )
                h16[a] = n_a
                if not last:
                    nc.gpsimd.dma_start(
                        o_ap[t, :, :PG], n_a[:]
                    )

                # PE: r-u-close b, r-opener b
                if not last:
                    nc.tensor.matmul(
                        zr_next[b][:, PG:], wslice(WRH), u_b[:],
                        start=False, stop=False, skip_group_check=True,
                    )
                    nc.tensor.matmul(
                        zr_next[b][:, PG:], wslice(WRX), x_next[:, PG:],
                        start=False, stop=False, skip_group_check=True,
                    )

                # blend b
                w_b = wk.tile([C, PG], f16, tag="w_1")
                nc.vector.scalar_tensor_tensor(
                    w_b[:], zbr[b][:, :PG], 1.0, c16b[:],
                    OP.subtract, OP.mult,
                )
                n_b = spool.tile([C, PG], f16, tag="h16_1")
                nc.vector.tensor_sub(n_b[:], u_b[:], w_b[:])
                if not last:
                    nc.tensor.matmul(
                        zr_next[b][:, PG:], wslice(WRHN), w_b[:],
                        start=False, stop=True, skip_group_check=True,
                    )
                    nc.tensor.matmul(
                        zr_next[b][:, :PG], wslice(WZXN), x_next[:, PG:],
                        start=False, stop=False, skip_group_check=True,
                    )
                    nc.tensor.matmul(
                        zr_next[b][:, :PG], wslice(WZHN), n_b[:],
                        start=False, stop=True, skip_group_check=True,
                    )
                h16[b] = n_b
                if not last:
                    nc.gpsimd.dma_start(
                        o_ap[t, :, PG:], n_b[:]
                    )

                # c openers for t+1 (PE tail; c tiles are single-buffered,
                # WAR on this step's tanh reads)
                cp_next = [None, None]
                if not last:
                    for g in (a, b):
                        cp = ps.tile([C, PG], f32, tag=f"c_{g}", bufs=1)
                        nc.tensor.matmul(
                            cp[:], wslice(WHX),
                            x_next[:, g * PG : (g + 1) * PG],
                            start=True, stop=False,
                        )
                        cp_next[g] = cp

                if last:
                    # final outputs ride the idle scalar+sync queues
                    hp = PG // 2
                    nc.scalar.dma_start(o_ap[t, :, :hp], n_a[:, :hp])
                    nc.sync.dma_start(o_ap[t, :, hp:PG], n_a[:, hp:])
                    nc.scalar.dma_start(
                        o_ap[t, :, PG : PG + hp], n_b[:, :hp]
                    )
                    nc.sync.dma_start(o_ap[t, :, PG + hp :], n_b[:, hp:])

                if t + 2 < T:
                    x_tiles[t + 2] = load_x(t + 2)
                x_tiles.pop(t - 1, None)
                if not last:
                    zr_t, cp_t = zr_next, cp_next

    nc.compile()
    return nc


def _get_prog():
    global _PROG
    if _PROG is None:
        _PROG = _build()
    return _PROG


def _make_in_maps(video, Wz, bz, Wr, br, Wh, bh):
    w7 = np.concatenate(
        [
            -Wz[:, :C].T, Wh[:, :C].T, Wr[:, :C].T,
            -Wz[:, C:].T, Wr[:, C:].T, Wh[:, C:].T,
            -Wr[:, C:].T,
        ],
        axis=1,
    ).astype(np.float16)
    b3 = np.stack([bz, br, bh, -bz], axis=1).astype(np.float32)
    in_maps = []
    for core in range(NCORES):
        b_, q = divmod(core, 4)
        xs = np.ascontiguousarray(
            video[b_, :, :, q * HQ : (q + 1) * HQ, :]
        ).reshape(T, C, P).astype(np.float16)
        in_maps.append({"x_seq": xs, "wmats": w7, "biases": b3})
    return in_maps


def kernel(video, Wz, bz, Wr, br, Wh, bh):
    _ensure_paths()
    from concourse.bass_utils import run_bass_kernel_spmd

    video = np.asarray(video, dtype=np.float32)
    nc = _get_prog()
    in_maps = _make_in_maps(video, Wz, bz, Wr, br, Wh, bh)
    res = run_bass_kernel_spmd(nc, in_maps, list(range(NCORES)))

    out = np.empty((B, T, C, H, W), np.float32)
    for core in range(NCORES):
        b_, q = divmod(core, 4)
        out[b_, :, :, q * HQ : (q + 1) * HQ, :] = np.asarray(
            res.results[core]["out_seq"]
        ).astype(np.float32).reshape(T, C, HQ, W)
    return out


# revision 7
# speedup vs baseline: 1.6839x; 1.6839x over previous
"""ConvGRU Trainium2 kernel.

video [B=2, T=16, C=128, H=64, W=64] f32; 1x1-conv GRU over T.
Sharding: data-parallel over (B x H/16) -> 8 cores, each core owns
P = 16*64 = 1024 pixels for all T; weights replicated.

Per core, per timestep (pixels on the free dim, channels on partitions):
    zr_pre = [Wzx@x + Wzh@h | Wrx@x + Wrh@h]      (PE, fp16 in / fp32 psum)
    z = sigmoid(zr_pre[:P] + bz); r = sigmoid(zr_pre[P:] + br)   (ACT)
    rh = r * h                                     (DVE)
    c = tanh(Whx@x + Whh@rh + bh)                  (PE + ACT)
    h' = u + v,  u = zbar*h,  v = z*c,  zbar = sigmoid(-pre_z)

G=2 pixel groups form two independent recurrence chains that
interleave on the engines.  The Scalar (ACT) engine is the pacing
resource: 6 sigmoid/tanh ops x ~690ns = 4.13us/step of streaming.
The remaining slack is the serial tail between the last tanh and the
next step's first r-sigmoid.  Structure choices that close it:

  - The next step's r-gate close is DISTRIBUTED over h' = u + v:
        pre_r(t+1) += Wrh@u(t)   (issues mid-step, u is ready early)
        pre_r(t+1) += Wrh@v(t)   (right after v -- the h' add leaves
                                  the sigmoid critical path entirely)
    The z-gate close stays a single Wzh@h' (zbar sits early in the
    next step's ACT stream, so it has slack).
  - Each group's zbar runs immediately after its own r-sigmoid, so
    that group's next-step zr openers (WAR on the single-buffered zr
    PSUM tile) clear the PE FIFO long before the r-closes arrive.
  - DVE tail is group-major (u,z,v,add per group): the first group's
    v/add never queue behind the second group's u/z, whose zbar lands
    later on the ACT stream.
  - t=0 shortcut: h0 == 0, so closers, r-sigmoid and rh are skipped.
  - fp16 everywhere: bf16 measures uniformly slower on this stack
    (ACTIVATE 687->823ns, TT 423->508ns); fp16 matmuls already
    pipeline at the 216ns/MM N=512 roofline.
  - DMA traffic is split across the two available HW queues (sync +
    gpsimd): per step one x prefetch (t+2 ahead) rides sync and the
    two h' output stores ride gpsimd -- together they are ~120GB/s,
    which saturates a single queue.  The last step's outputs split
    across the scalar+sync queues instead (the ACT queue is idle by
    then and the gpsimd dge_drain would add ~3.6us of tail).
  - Startup DMAs are balanced across both queues (x0/x1 half-tiles,
    x-side weights on sync / h-side on gpsimd) because the t0/t1
    ramp is landing-bandwidth-bound (~60-70GB/s per queue).
  - PE warmup matmuls run against a memset tile (no weight-DMA
    dependency) flipping the HAM clock gate during the initial DMAs.

Measured: 84.2-85.7us (median 85.3) vs the 93.1us v1 baseline;
steady-state period 4233ns/step vs the 4128ns ACT-streaming floor.

Numerics: fp16 matmul inputs/gates/state, fp32 PSUM accum + fp32 bias.
"""

import os
import sys

import numpy as np

B, T, C, H, W = 2, 16, 128, 64, 64
NCORES = 8
HQ = H // 4          # 16 rows of H per core (4 H-slices x 2 batches = 8 cores)
P = HQ * W           # 1024 pixels per core
G = 2                # pixel groups per step (independent recurrence chains)
PG = P // G          # 512 pixels per group

_PROG = None


def _ensure_paths():
    for p in ("/opt/trn_rl_repo",):
        if p not in sys.path and os.path.isdir(p):
            sys.path.append(p)


def _build():
    _ensure_paths()
    import concourse.bacc as bacc
    import concourse.tile as tile
    from concourse import mybir

    f32 = mybir.dt.float32
    f16 = mybir.dt.float16
    AF = mybir.ActivationFunctionType

    nc = bacc.Bacc(
        "TRN2", target_bir_lowering=False, debug=False, num_devices=NCORES
    )
    x_dram = nc.dram_tensor("x_seq", [T, C, P], f16, kind="ExternalInput")
    w_dram = nc.dram_tensor("wmats", [C, 6 * C], f16, kind="ExternalInput")
    b_dram = nc.dram_tensor("biases", [C, 4], f32, kind="ExternalInput")
    o_dram = nc.dram_tensor("out_seq", [T, C, P], f16, kind="ExternalOutput")

    x_ap = x_dram.ap()
    w_ap = w_dram.ap()
    b_ap = b_dram.ap()
    o_ap = o_dram.ap()

    # weight order in wmats: x-side first so its DMA can land first
    WZX, WHX, WRX, WZH, WRH, WHH = range(6)

    with tile.TileContext(nc) as tc:
        with (
            tc.tile_pool(name="consts", bufs=1) as consts,
            tc.tile_pool(name="xin", bufs=4) as xpool,
            tc.tile_pool(name="state", bufs=4) as spool,
            tc.tile_pool(name="work", bufs=3) as wk,
            tc.tile_pool(name="ps", bufs=1, space="PSUM") as ps,
        ):
            wt = consts.tile([C, 6 * C], f16)
            bt = consts.tile([C, 4], f32)
            nc.sync.dma_start(bt[:], b_ap[:])

            def wslice(i):
                return wt[:, i * C : (i + 1) * C]

            def load_x(t):
                xt = xpool.tile([C, P], f16, tag="x")
                nc.sync.dma_start(xt[:], x_ap[t])
                return xt

            # startup DMA spread over the three queues (sync/scalar HWDGE +
            # gpsimd SWDGE), ordered by first use: x0 quarters ride the two
            # HWDGE queues so t0 can start ~2us earlier; t0-only weights
            # (Wzx|Whx) go first on gpsimd; the t1 weights ride sync behind
            # the x0 quarters.
            Q = PG // 2
            x0t = xpool.tile([C, P], f16, tag="x", name="x0t")
            x1t = xpool.tile([C, P], f16, tag="x", name="x1t")
            nc.sync.dma_start(x0t[:, 0:Q], x_ap[0, :, 0:Q])
            nc.sync.dma_start(x0t[:, Q : 2 * Q], x_ap[0, :, Q : 2 * Q])
            nc.scalar.dma_start(x0t[:, 2 * Q : 3 * Q], x_ap[0, :, 2 * Q : 3 * Q])
            nc.sync.dma_start(x0t[:, 3 * Q :], x_ap[0, :, 3 * Q :])
            nc.gpsimd.dma_start(wt[:, : 2 * C], w_ap[:, : 2 * C])
            nc.sync.dma_start(wt[:, 2 * C :], w_ap[:, 2 * C :])
            nc.gpsimd.dma_start(x1t[:, :PG], x_ap[1, :, :PG])
            nc.gpsimd.dma_start(x1t[:, PG:], x_ap[1, :, PG:])
            x_tiles = {0: x0t, 1: x1t}
            x_tiles[2] = load_x(2)

            # -- warmup: ramp the PE clock gate with matmuls that only
            #    depend on a memset tile, while the input DMAs fly --
            warm16 = wk.tile([C, PG], f16, tag="warm")
            nc.vector.memset(warm16[:], 0.0)
            cwarm = [None, None]
            for g in range(G):
                cwarm[g] = ps.tile(
                    [C, PG], f32, tag=f"c_{g}", bufs=2, name=f"cwarm_{g}"
                )
            for i in range(6):
                nc.tensor.matmul(
                    cwarm[i % 2][:], warm16[:, :C], warm16[:],
                    start=True, stop=True,
                )
            # preload the ACT sigmoid/tanh table early; const bias so the
            # table load has no DMA dependency and rides the ramp
            wtmp = wk.tile([C, PG], f16, tag="scratch")
            nc.scalar.activation(wtmp[:], cwarm[0][:], AF.Sigmoid)

            def open_zr(xt, g):
                """Open one group's z|r accumulation with the x-side."""
                xs = xt[:, g * PG : (g + 1) * PG]
                zrt = ps.tile([C, 2 * PG], f32, tag=f"zr_{g}", bufs=1,
                              name=f"zr_t{g}")
                nc.tensor.matmul(
                    zrt[:, PG:], wslice(WRX), xs, start=True, stop=False
                )
                nc.tensor.matmul(
                    zrt[:, :PG], wslice(WZX), xs, start=True, stop=False
                )
                return zrt

            def open_c(xt, g):
                xs = xt[:, g * PG : (g + 1) * PG]
                cp = ps.tile([C, PG], f32, tag=f"c_{g}", bufs=2,
                             name=f"c_t{g}")
                nc.tensor.matmul(
                    cp[:], wslice(WHX), xs, start=True, stop=False
                )
                return cp

            # ---- t = 0: h0 == 0, so no closers / r-gate / rh ----
            x0 = x_tiles[0]
            zr0 = [None, None]
            for g in range(G):
                zrt = ps.tile([C, 2 * PG], f32, tag=f"zr_{g}", name=f"zr0_{g}")
                nc.tensor.matmul(
                    zrt[:, :PG], wslice(WZX), x0[:, g * PG : (g + 1) * PG],
                    start=True, stop=True,
                )
                zr0[g] = zrt
            c0 = [None, None]
            for g in range(G):
                cp = ps.tile([C, PG], f32, tag=f"c_{g}", bufs=2)
                nc.tensor.matmul(
                    cp[:], wslice(WHX), x0[:, g * PG : (g + 1) * PG],
                    start=True, stop=True,
                )
                c0[g] = cp
            h16 = [None, None]
            for g in range(G):
                zt = wk.tile([C, PG], f16, tag=f"zb_{g}")
                nc.scalar.activation(
                    zt[:], zr0[g][:, :PG], AF.Sigmoid, bias=bt[:, 0:1]
                )
                ct = wk.tile([C, PG], f16, tag=f"c16_{g}")
                nc.scalar.activation(ct[:], c0[g][:], AF.Tanh, bias=bt[:, 2:3])
                ht = spool.tile([C, PG], f16, tag=f"h16_{g}")
                nc.vector.tensor_mul(ht[:], zt[:], ct[:])
                h16[g] = ht
                nc.sync.dma_start(
                    o_ap[0, :, g * PG : (g + 1) * PG], ht[:]
                )

            # open + close t=1's zr with h'(0) (plain single closes)
            x1 = x_tiles[1]
            zr_t = [None, None]
            cp_t = [None, None]
            for g in range(G):
                zr_t[g] = open_zr(x1, g)
                nc.tensor.matmul(
                    zr_t[g][:, PG:], wslice(WRH), h16[g][:],
                    start=False, stop=True,
                )
                nc.tensor.matmul(
                    zr_t[g][:, :PG], wslice(WZH), h16[g][:],
                    start=False, stop=True,
                )
                cp_t[g] = open_c(x1, g)

            # ---- steady steps t = 1..T-1 ----
            # zr_t arrives FULLY CLOSED (r closed via Wrh@u + Wrh@v of the
            # previous step's blend; z closed via Wzh@h')
            for t in range(1, T):
                go = [0, 1] if t % 2 == 1 else [1, 0]
                a, b = go
                x_next = x_tiles.get(t + 1)
                if t + 2 < T:
                    x_tiles[t + 2] = load_x(t + 2)

                r16, zb16 = [None] * G, [None] * G

                def sig_r(g):
                    rt = wk.tile([C, PG], f16, tag=f"r_{g}", name=f"r16_{g}")
                    nc.scalar.activation(
                        rt[:], zr_t[g][:, PG:], AF.Sigmoid, bias=bt[:, 1:2]
                    )
                    r16[g] = rt

                def sig_zbar(g):
                    zbt = wk.tile([C, PG], f16, tag=f"zb_{g}", name=f"zb16_{g}")
                    nc.scalar.activation(
                        zbt[:], zr_t[g][:, :PG], AF.Sigmoid,
                        bias=bt[:, 3:4], scale=-1.0,
                    )
                    zb16[g] = zbt

                sig_r(a)
                sig_zbar(a)
                sig_r(b)

                # group a's next-step z|r openers (zr_a fully consumed)
                zr_next = [None] * G
                if x_next is not None:
                    zr_next[a] = open_zr(x_next, a)

                rh16 = [None] * G
                for g in go:
                    rh = wk.tile([C, PG], f16, tag=f"rh_{g}")
                    nc.vector.tensor_mul(rh[:], r16[g][:], h16[g][:])
                    rh16[g] = rh

                for g in go:
                    nc.tensor.matmul(
                        cp_t[g][:], wslice(WHH), rh16[g][:],
                        start=False, stop=True,
                    )

                cp_next = [None] * G

                c16 = [None] * G

                def tanh_c(g):
                    ct = wk.tile([C, PG], f16, tag=f"c16_{g}", name=f"c16_{g}")
                    nc.scalar.activation(
                        ct[:], cp_t[g][:], AF.Tanh, bias=bt[:, 2:3]
                    )
                    c16[g] = ct

                u16, z16 = [None] * G, [None] * G

                def blend_pre(g):
                    """u,z depend only on zbar; on the last step they are
                    hoisted before the tanh so the final h'-adds start the
                    moment the tanh lands."""
                    ut = wk.tile([C, PG], f16, tag=f"u_{g}", name=f"u16_{g}")
                    nc.vector.tensor_mul(ut[:], zb16[g][:], h16[g][:])
                    zt = wk.tile([C, PG], f16, tag=f"z_{g}", name=f"z16_{g}")
                    nc.vector.tensor_scalar(
                        zt[:], zb16[g][:], -1.0, 1.0,
                        mybir.AluOpType.mult, mybir.AluOpType.add,
                    )
                    u16[g], z16[g] = ut, zt
                    if zr_next[g] is not None:
                        nc.tensor.matmul(
                            zr_next[g][:, PG:], wslice(WRH), ut[:],
                            start=False, stop=False,
                        )

                def blend(g):
                    """v + h'-add after tanh; the next step's r-close rides
                    u and v so the sigmoid path never waits for the add."""
                    ut, zt = u16[g], z16[g]
                    v16 = wk.tile([C, PG], f16, tag=f"v_{g}", name=f"v16_{g}")
                    nc.vector.tensor_mul(v16[:], zt[:], c16[g][:])
                    if zr_next[g] is not None:
                        with tc.high_priority(offset=25):
                            nc.tensor.matmul(
                                zr_next[g][:, PG:], wslice(WRH), v16[:],
                                start=False, stop=True,
                            )
                    n16 = spool.tile([C, PG], f16, tag=f"h16_{g}",
                                     name=f"h16n_{g}")
                    if t + 1 < T:
                        nc.vector.tensor_add(n16[:], ut[:], v16[:])
                        h16[g] = n16
                        nc.tensor.matmul(
                            zr_next[g][:, :PG], wslice(WZH), n16[:],
                            start=False, stop=True,
                        )
                        nc.gpsimd.dma_start(
                            o_ap[t, :, g * PG : (g + 1) * PG], n16[:]
                        )
                    else:
                        # final step: chunk the add so each half's store
                        # dispatches as soon as that half is summed
                        hp = PG // 2
                        nc.vector.tensor_add(
                            n16[:, :hp], ut[:, :hp], v16[:, :hp]
                        )
                        nc.scalar.dma_start(
                            o_ap[t, :, g * PG : g * PG + hp], n16[:, :hp]
                        )
                        nc.vector.tensor_add(
                            n16[:, hp:], ut[:, hp:], v16[:, hp:]
                        )
                        nc.sync.dma_start(
                            o_ap[t, :, g * PG + hp : (g + 1) * PG], n16[:, hp:]
                        )
                        h16[g] = n16

                last = x_next is None
                if last:
                    blend_pre(a)
                tanh_c(a)
                sig_zbar(b)
                if not last:
                    blend_pre(a)
                blend(a)
                if not last:
                    for g in go:
                        cp_next[g] = open_c(x_next, g)
                if last:
                    blend_pre(b)
                tanh_c(b)
                if not last:
                    zr_next[b] = open_zr(x_next, b)
                    blend_pre(b)
                blend(b)

                x_tiles.pop(t - 1, None)
                if x_next is not None:
                    zr_t, cp_t = zr_next, cp_next

    nc.compile()
    return nc


def _get_prog():
    global _PROG
    if _PROG is None:
        _PROG = _build()
    return _PROG


def _make_in_maps(video, Wz, bz, Wr, br, Wh, bh):
    w6 = np.concatenate(
        [
            Wz[:, :C].T, Wh[:, :C].T, Wr[:, :C].T,
            Wz[:, C:].T, Wr[:, C:].T, Wh[:, C:].T,
        ],
        axis=1,
    ).astype(np.float16)
    b3 = np.stack([bz, br, bh, -bz], axis=1).astype(np.float32)
    in_maps = []
    for core in range(NCORES):
        b_, q = divmod(core, 4)
        xs = np.ascontiguousarray(
            video[b_, :, :, q * HQ : (q + 1) * HQ, :]
        ).reshape(T, C, P).astype(np.float16)
        in_maps.append({"x_seq": xs, "wmats": w6, "biases": b3})
    return in_maps


def kernel(video, Wz, bz, Wr, br, Wh, bh):
    _ensure_paths()
    from concourse.bass_utils import run_bass_kernel_spmd

    video = np.asarray(video, dtype=np.float32)
    nc = _get_prog()
    in_maps = _make_in_maps(video, Wz, bz, Wr, br, Wh, bh)
    res = run_bass_kernel_spmd(nc, in_maps, list(range(NCORES)))

    out = np.empty((B, T, C, H, W), np.float32)
    for core in range(NCORES):
        b_, q = divmod(core, 4)
        out[b_, :, :, q * HQ : (q + 1) * HQ, :] = np.asarray(
            res.results[core]["out_seq"]
        ).astype(np.float32).reshape(T, C, HQ, W)
    return out



# revision 11
# speedup vs baseline: 1.7410x; 1.0340x over previous
"""ConvGRU Trainium2 kernel.

video [B=2, T=16, C=128, H=64, W=64] f32; 1x1-conv GRU over T.
Sharding: data-parallel over (B x H/16) -> 8 cores, each core owns
P = 16*64 = 1024 pixels for all T; weights replicated.

Per core, per timestep (pixels on the free dim, channels on partitions):
    zr_pre = [Wzx@x + Wzh@h | Wrx@x + Wrh@h]      (PE, fp16 in / fp32 psum)
    z = sigmoid(zr_pre[:P] + bz); r = sigmoid(zr_pre[P:] + br)   (ACT)
    rh = r * h                                     (DVE)
    c = tanh(Whx@x + Whh@rh + bh)                  (PE + ACT)
    h' = u + v,  u = zbar*h,  v = z*c,  zbar = sigmoid(-pre_z)

G=2 pixel groups form two independent recurrence chains that
interleave on the engines.  The Scalar (ACT) engine is the pacing
resource: 6 sigmoid/tanh ops x ~690ns = 4.13us/step of streaming.
The remaining slack is the serial tail between the last tanh and the
next step's first r-sigmoid.  Structure choices that close it:

  - The next step's r-gate close is DISTRIBUTED over h' = u + v:
        pre_r(t+1) += Wrh@u(t)   (issues mid-step, u is ready early)
        pre_r(t+1) += Wrh@v(t)   (right after v -- the h' add leaves
                                  the sigmoid critical path entirely)
    The z-gate close stays a single Wzh@h' (zbar sits early in the
    next step's ACT stream, so it has slack).
  - Each group's zbar runs immediately after its own r-sigmoid, so
    that group's next-step zr openers (WAR on the single-buffered zr
    PSUM tile) clear the PE FIFO long before the r-closes arrive.
  - DVE tail is group-major (u,z,v,add per group): the first group's
    v/add never queue behind the second group's u/z, whose zbar lands
    later on the ACT stream.
  - t=0 shortcut: h0 == 0, so closers, r-sigmoid and rh are skipped.
  - fp16 everywhere: bf16 measures uniformly slower on this stack
    (ACTIVATE 687->823ns, TT 423->508ns); fp16 matmuls already
    pipeline at the 216ns/MM N=512 roofline.
  - DMA traffic is split across the two available HW queues (sync +
    gpsimd): per step one x prefetch (t+2 ahead) rides sync and the
    two h' output stores ride gpsimd -- together they are ~120GB/s,
    which saturates a single queue.  The last step's outputs split
    across the scalar+sync queues instead (the ACT queue is idle by
    then and the gpsimd dge_drain would add ~3.6us of tail).
  - Startup DMAs are balanced across both queues (x0/x1 half-tiles,
    x-side weights on sync / h-side on gpsimd) because the t0/t1
    ramp is landing-bandwidth-bound (~60-70GB/s per queue).
  - PE warmup matmuls run against a memset tile (no weight-DMA
    dependency) flipping the HAM clock gate during the initial DMAs.

Measured: 84.2-85.7us (median 85.3) vs the 93.1us v1 baseline;
steady-state period 4233ns/step vs the 4128ns ACT-streaming floor.

Numerics: fp16 matmul inputs/gates/state, fp32 PSUM accum + fp32 bias.
"""

import os
import sys

import numpy as np

B, T, C, H, W = 2, 16, 128, 64, 64
NCORES = 8
HQ = H // 4          # 16 rows of H per core (4 H-slices x 2 batches = 8 cores)
P = HQ * W           # 1024 pixels per core
G = 2                # pixel groups per step (independent recurrence chains)
PG = P // G          # 512 pixels per group

_PROG = None


def _ensure_paths():
    for p in ("/opt/trn_rl_repo",):
        if p not in sys.path and os.path.isdir(p):
            sys.path.append(p)


def _build():
    _ensure_paths()
    import concourse.bacc as bacc
    import concourse.tile as tile
    from concourse import mybir

    f32 = mybir.dt.float32
    f16 = mybir.dt.float16
    AF = mybir.ActivationFunctionType

    nc = bacc.Bacc(
        "TRN2", target_bir_lowering=False, debug=False, num_devices=NCORES
    )
    x_dram = nc.dram_tensor("x_seq", [T, C, P], f16, kind="ExternalInput")
    w_dram = nc.dram_tensor("wmats", [C, 6 * C], f16, kind="ExternalInput")
    b_dram = nc.dram_tensor("biases", [C, 4], f32, kind="ExternalInput")
    o_dram = nc.dram_tensor("out_seq", [T, C, P], f16, kind="ExternalOutput")

    x_ap = x_dram.ap()
    w_ap = w_dram.ap()
    b_ap = b_dram.ap()
    o_ap = o_dram.ap()

    # weight order in wmats: x-side first so its DMA can land first
    WZX, WHX, WRX, WZH, WRH, WHH = range(6)

    with tile.TileContext(nc) as tc:
        with (
            tc.tile_pool(name="consts", bufs=1) as consts,
            tc.tile_pool(name="xin", bufs=4) as xpool,
            tc.tile_pool(name="state", bufs=4) as spool,
            tc.tile_pool(name="work", bufs=3) as wk,
            tc.tile_pool(name="ps", bufs=1, space="PSUM") as ps,
        ):
            # weights live in two tiles so t0's matmuls only gate on the
            # (Wzx|Whx) DMA, not on the whole weight load; likewise x0/x1
            # are per-group half tiles so each group's openers gate on
            # their own half.
            wt_lo = consts.tile([C, 2 * C], f16)
            wt_mid = consts.tile([C, 4 * C], f16)
            bt = consts.tile([C, 4], f32)
            nc.sync.dma_start(bt[:], b_ap[:])

            def wslice(i):
                if i < 2:
                    return wt_lo[:, i * C : (i + 1) * C]
                return wt_mid[:, (i - 2) * C : (i - 1) * C]

            def load_x(t):
                xt = xpool.tile([C, P], f16, tag="x")
                nc.sync.dma_start(xt[:], x_ap[t])
                return xt

            # startup DMA, ordered by first use.  sync (fast HWDGE): bt,
            # group-a x0 half, t1 weights, x2.  gpsimd (fast SWDGE): t0
            # weights, group-b x0 half, x1a.  scalar (slow HWDGE): x1b
            # only, and dispatched after the warmup sigmoid so the ACT
            # table load isn't split in two.
            x0h = [xpool.tile([C, PG], f16, tag="x0a", name="x0a"),
                   xpool.tile([C, PG], f16, tag="x0b", name="x0b")]
            x1h = [xpool.tile([C, PG], f16, tag="x1a", name="x1a"),
                   xpool.tile([C, PG], f16, tag="x1b", name="x1b")]
            nc.sync.dma_start(x0h[0][:], x_ap[0, :, :PG])
            nc.gpsimd.dma_start(wt_lo[:], w_ap[:, : 2 * C])
            nc.gpsimd.dma_start(x0h[1][:], x_ap[0, :, PG:])
            nc.sync.dma_start(wt_mid[:], w_ap[:, 2 * C :])
            nc.gpsimd.dma_start(x1h[0][:], x_ap[1, :, :PG])
            x_tiles = {}
            x_tiles[2] = load_x(2)

            # -- warmup: ramp the PE clock gate with matmuls that only
            #    depend on a memset tile, while the input DMAs fly --
            warm16 = wk.tile([C, PG], f16, tag="warm")
            nc.vector.memset(warm16[:], 0.0)
            cwarm = [None, None]
            for g in range(G):
                cwarm[g] = ps.tile(
                    [C, PG], f32, tag=f"c_{g}", bufs=2, name=f"cwarm_{g}"
                )
            for i in range(6):
                nc.tensor.matmul(
                    cwarm[i % 2][:], warm16[:, :C], warm16[:],
                    start=True, stop=True,
                )
            # preload the ACT sigmoid/tanh table early; const bias so the
            # table load has no DMA dependency and rides the ramp
            wtmp = wk.tile([C, PG], f16, tag="scratch")
            nc.scalar.activation(wtmp[:], cwarm[0][:], AF.Sigmoid)
            # slow scalar queue carries only the slack x1 half; dispatched
            # after the warmup sigmoid so the table load isn't split
            nc.scalar.dma_start(x1h[1][:], x_ap[1, :, PG:])

            def open_zr(xs, g):
                """Open one group's z|r accumulation with the x-side."""
                zrt = ps.tile([C, 2 * PG], f32, tag=f"zr_{g}", bufs=1,
                              name=f"zr_t{g}")
                nc.tensor.matmul(
                    zrt[:, PG:], wslice(WRX), xs, start=True, stop=False
                )
                nc.tensor.matmul(
                    zrt[:, :PG], wslice(WZX), xs, start=True, stop=False
                )
                return zrt

            def open_c(xs, g):
                cp = ps.tile([C, PG], f32, tag=f"c_{g}", bufs=2,
                             name=f"c_t{g}")
                nc.tensor.matmul(
                    cp[:], wslice(WHX), xs, start=True, stop=False
                )
                return cp

            # ---- t = 0: h0 == 0, so no closers / r-gate / rh ----
            zr0 = [None, None]
            for g in range(G):
                zrt = ps.tile([C, 2 * PG], f32, tag=f"zr_{g}", name=f"zr0_{g}")
                nc.tensor.matmul(
                    zrt[:, :PG], wslice(WZX), x0h[g][:],
                    start=True, stop=True,
                )
                zr0[g] = zrt
            c0 = [None, None]
            for g in range(G):
                cp = ps.tile([C, PG], f32, tag=f"c_{g}", bufs=2)
                nc.tensor.matmul(
                    cp[:], wslice(WHX), x0h[g][:],
                    start=True, stop=True,
                )
                c0[g] = cp
            h16 = [None, None]
            for g in range(G):
                zt = wk.tile([C, PG], f16, tag=f"zb_{g}")
                nc.scalar.activation(
                    zt[:], zr0[g][:, :PG], AF.Sigmoid, bias=bt[:, 0:1]
                )
                ct = wk.tile([C, PG], f16, tag=f"c16_{g}")
                nc.scalar.activation(ct[:], c0[g][:], AF.Tanh, bias=bt[:, 2:3])
                ht = spool.tile([C, PG], f16, tag=f"h16_{g}")
                nc.vector.tensor_mul(ht[:], zt[:], ct[:])
                h16[g] = ht
                nc.sync.dma_start(
                    o_ap[0, :, g * PG : (g + 1) * PG], ht[:]
                )

            # open + close t=1's zr with h'(0) (plain single closes)
            zr_t = [None, None]
            cp_t = [None, None]
            for g in range(G):
                zr_t[g] = open_zr(x1h[g][:], g)
                nc.tensor.matmul(
                    zr_t[g][:, PG:], wslice(WRH), h16[g][:],
                    start=False, stop=True,
                )
                nc.tensor.matmul(
                    zr_t[g][:, :PG], wslice(WZH), h16[g][:],
                    start=False, stop=True,
                )
                cp_t[g] = open_c(x1h[g][:], g)

            # ---- steady steps t = 1..T-1 ----
            # zr_t arrives FULLY CLOSED (r closed via Wrh@u + Wrh@v of the
            # previous step's blend; z closed via Wzh@h')
            for t in range(1, T):
                go = [0, 1] if t % 2 == 1 else [1, 0]
                a, b = go
                x_next = x_tiles.get(t + 1)
                if t + 2 < T:
                    x_tiles[t + 2] = load_x(t + 2)

                r16, zb16 = [None] * G, [None] * G

                def sig_r(g):
                    rt = wk.tile([C, PG], f16, tag=f"r_{g}", name=f"r16_{g}")
                    nc.scalar.activation(
                        rt[:], zr_t[g][:, PG:], AF.Sigmoid, bias=bt[:, 1:2]
                    )
                    r16[g] = rt

                def sig_zbar(g):
                    zbt = wk.tile([C, PG], f16, tag=f"zb_{g}", name=f"zb16_{g}")
                    nc.scalar.activation(
                        zbt[:], zr_t[g][:, :PG], AF.Sigmoid,
                        bias=bt[:, 3:4], scale=-1.0,
                    )
                    zb16[g] = zbt

                sig_r(a)
                sig_zbar(a)
                sig_r(b)

                # group a's next-step z|r openers (zr_a fully consumed)
                zr_next = [None] * G
                if x_next is not None:
                    zr_next[a] = open_zr(x_next[:, a * PG : (a + 1) * PG], a)

                rh16 = [None] * G
                for g in go:
                    rh = wk.tile([C, PG], f16, tag=f"rh_{g}")
                    nc.vector.tensor_mul(rh[:], r16[g][:], h16[g][:])
                    rh16[g] = rh

                for g in go:
                    nc.tensor.matmul(
                        cp_t[g][:], wslice(WHH), rh16[g][:],
                        start=False, stop=True,
                    )

                cp_next = [None] * G

                c16 = [None] * G

                def tanh_c(g):
                    ct = wk.tile([C, PG], f16, tag=f"c16_{g}", name=f"c16_{g}")
                    nc.scalar.activation(
                        ct[:], cp_t[g][:], AF.Tanh, bias=bt[:, 2:3]
                    )
                    c16[g] = ct

                u16, z16 = [None] * G, [None] * G

                def blend_pre(g):
                    """u,z depend only on zbar; on the last step they are
                    hoisted before the tanh so the final h'-adds start the
                    moment the tanh lands."""
                    ut = wk.tile([C, PG], f16, tag=f"u_{g}", name=f"u16_{g}")
                    nc.vector.tensor_mul(ut[:], zb16[g][:], h16[g][:])
                    zt = wk.tile([C, PG], f16, tag=f"z_{g}", name=f"z16_{g}")
                    nc.vector.tensor_scalar(
                        zt[:], zb16[g][:], -1.0, 1.0,
                        mybir.AluOpType.mult, mybir.AluOpType.add,
                    )
                    u16[g], z16[g] = ut, zt
                    if zr_next[g] is not None:
                        nc.tensor.matmul(
                            zr_next[g][:, PG:], wslice(WRH), ut[:],
                            start=False, stop=False,
                        )

                def blend(g):
                    """v + h'-add after tanh; the next step's r-close rides
                    u and v so the sigmoid path never waits for the add."""
                    ut, zt = u16[g], z16[g]
                    v16 = wk.tile([C, PG], f16, tag=f"v_{g}", name=f"v16_{g}")
                    nc.vector.tensor_mul(v16[:], zt[:], c16[g][:])
                    if zr_next[g] is not None:
                        with tc.high_priority(offset=25):
                            nc.tensor.matmul(
                                zr_next[g][:, PG:], wslice(WRH), v16[:],
                                start=False, stop=True,
                            )
                    n16 = spool.tile([C, PG], f16, tag=f"h16_{g}",
                                     name=f"h16n_{g}")
                    if t + 1 < T:
                        nc.vector.tensor_add(n16[:], ut[:], v16[:])
                        h16[g] = n16
                        nc.tensor.matmul(
                            zr_next[g][:, :PG], wslice(WZH), n16[:],
                            start=False, stop=True,
                        )
                        nc.gpsimd.dma_start(
                            o_ap[t, :, g * PG : (g + 1) * PG], n16[:]
                        )
                    else:
                        # final step: chunk the add so each half's store
                        # dispatches as soon as that half is summed
                        hp = PG // 2
                        nc.vector.tensor_add(
                            n16[:, :hp], ut[:, :hp], v16[:, :hp]
                        )
                        nc.scalar.dma_start(
                            o_ap[t, :, g * PG : g * PG + hp], n16[:, :hp]
                        )
                        nc.vector.tensor_add(
                            n16[:, hp:], ut[:, hp:], v16[:, hp:]
                        )
                        nc.sync.dma_start(
                            o_ap[t, :, g * PG + hp : (g + 1) * PG], n16[:, hp:]
                        )
                        h16[g] = n16

                last = x_next is None
                if last:
                    blend_pre(a)
                tanh_c(a)
                sig_zbar(b)
                if not last:
                    blend_pre(a)
                blend(a)
                if not last:
                    for g in go:
                        cp_next[g] = open_c(x_next[:, g * PG : (g + 1) * PG], g)
                if last:
                    blend_pre(b)
                tanh_c(b)
                if not last:
                    zr_next[b] = open_zr(x_next[:, b * PG : (b + 1) * PG], b)
                    blend_pre(b)
                blend(b)

                x_tiles.pop(t - 1, None)
                if x_next is not None:
                    zr_t, cp_t = zr_next, cp_next

    nc.compile()
    return nc


def _get_prog():
    global _PROG
    if _PROG is None:
        _PROG = _build()
    return _PROG


def _make_in_maps(video, Wz, bz, Wr, br, Wh, bh):
    w6 = np.concatenate(
        [
            Wz[:, :C].T, Wh[:, :C].T, Wr[:, :C].T,
            Wz[:, C:].T, Wr[:, C:].T, Wh[:, C:].T,
        ],
        axis=1,
    ).astype(np.float16)
    b3 = np.stack([bz, br, bh, -bz], axis=1).astype(np.float32)
    in_maps = []
    for core in range(NCORES):
        b_, q = divmod(core, 4)
        xs = np.ascontiguousarray(
            video[b_, :, :, q * HQ : (q + 1) * HQ, :]
        ).reshape(T, C, P).astype(np.float16)
        in_maps.append({"x_seq": xs, "wmats": w6, "biases": b3})
    return in_maps


def kernel(video, Wz, bz, Wr, br, Wh, bh):
    _ensure_paths()
    from concourse.bass_utils import run_bass_kernel_spmd

    video = np.asarray(video, dtype=np.float32)
    nc = _get_prog()
    in_maps = _make_in_maps(video, Wz, bz, Wr, br, Wh, bh)
    res = run_bass_kernel_spmd(nc, in_maps, list(range(NCORES)))

    out = np.empty((B, T, C, H, W), np.float32)
    for core in range(NCORES):
        b_, q = divmod(core, 4)
        out[b_, :, :, q * HQ : (q + 1) * HQ, :] = np.asarray(
            res.results[core]["out_seq"]
        ).astype(np.float32).reshape(T, C, HQ, W)
    return out

